# revision 37
# baseline (speedup 1.0000x reference)
"""Causal multi-head attention (B=2, T=2048, C=1024, H=16) on 8 TRN2 cores.

Sharding: data-parallel over batch (2 groups of 4 cores), tensor-parallel
over heads within a group (4 heads / core). Each core:
  1. computes Q^T, K^T (layout [d, t]) and V (layout [t, d], bias folded
     in) for its heads from x[b]^T and its W column slices,
  2. runs causal attention in the S^T = K @ Q^T orientation; softmax sums
     come from a ones-column appended to V; exp is batched over both heads
     of a pair (one ACT per k-step over a 2-bank PSUM tile); diagonal
     k-tiles are trimmed to q >= 128*dm,
  3. AllGathers the per-head attention outputs per q-chunk across the 4
     cores of its batch group,
  4. computes a 256-column slice of the output projection per q-chunk.
Host reassembles the 8 [2048, 256] shards into [2, 2048, 1024].

Scheduling: projection chains that are not needed upfront are emitted as
"fillers" between attention steps so the PE stream stays dense while the
scalar engine (exp) is the per-step bottleneck.
"""

import os
import sys

import numpy as np
import ml_dtypes

for _p in ("/opt/trn_rl_repo",):
    if os.path.isdir(_p) and _p not in sys.path:
        sys.path.insert(0, _p)

import concourse.bacc as bacc
import concourse.mybir as mybir
import concourse.tile as tile
from concourse import bass_utils

B, T, C, H, D = 2, 2048, 1024, 16, 64
NCORES = 8
GP = 4              # cores per batch group
HPC = H // GP       # heads per core = 4
DS = HPC * D        # per-core head-dim slice = 256
NCT = C // 128      # c-tiles = 8
NQC = T // 512      # q-chunks = 4
NKT = T // 128      # k-tiles = 16
VW = NKT * 65       # vp row width per head (64 V cols + ones col per k-tile)
BISECT_NO_FILLERS = False

F32 = mybir.dt.float32
F32R = mybir.dt.float32r
BF16 = mybir.dt.bfloat16
AF = mybir.ActivationFunctionType
ALU = mybir.AluOpType
NPBF = ml_dtypes.bfloat16

_PROG = None
LAST_RESULTS = None  # BassKernelResults of the most recent run (for test.py)


def _r(ap):
    return ap.bitcast(F32R)


def _emit(nc, tc, io):
    (xT, wq, wk, wv, wo, bq2, bk2, bv_bc, bo_bc, maskd, onesd,
     out_shard) = io

    agi01 = nc.dram_tensor("agi01", [DS, 1024], BF16)
    ago01 = nc.dram_tensor(
        "ago01", [NCORES * DS, 1024], BF16, addr_space="Shared"
    )
    agi2 = nc.dram_tensor("agi2", [DS, 512], BF16)
    ago2 = nc.dram_tensor(
        "ago2", [NCORES * DS, 512], BF16, addr_space="Shared"
    )
    agi3 = [nc.dram_tensor(f"agi3{p}", [128, 512], BF16) for p in range(2)]
    ago3 = [
        nc.dram_tensor(f"ago3{p}", [NCORES * 128, 512], BF16, addr_space="Shared")
        for p in range(2)
    ]
    GROUPS = [list(range(NCORES))]

    with (
        tc.tile_pool(name="per", bufs=1) as po,
        tc.tile_pool(name="pT", bufs=4) as pw,
        tc.tile_pool(name="nrm", bufs=4) as pn,
        tc.tile_pool(name="osb", bufs=3) as posb,
        tc.tile_pool(name="pao", bufs=2) as pao,
        tc.tile_pool(name="psS", bufs=1, space="PSUM") as psS,
        tc.tile_pool(name="psO", bufs=1, space="PSUM") as psO,
        tc.tile_pool(name="psF", bufs=2, space="PSUM") as psF,
    ):
        # ---- input loads, spread over per-engine DMA queues -------------
        # sync: xT in column halves (first half unblocks qc0 work early)
        xT_sb = []
        for ci in range(NCT):
            t_ = po.tile([128, T], BF16, tag=f"xt{ci}", name=f"xt{ci}")
            xT_sb.append(t_)
        for th in range(2):
            for ci in range(NCT):
                nc.sync.dma_start(
                    xT_sb[ci][:, 1024 * th : 1024 * (th + 1)],
                    xT[128 * ci : 128 * (ci + 1), 1024 * th : 1024 * (th + 1)],
                )
        # scalar queue: wq, wk + biases
        wq_sb, wk_sb, wv_sb, wo_sb = [], [], [], []
        bq_sb = po.tile([128, 2], F32, tag="bq")
        nc.scalar.dma_start(bq_sb[:, :], bq2[:, :])
        bk_sb = po.tile([128, 2], F32, tag="bk")
        nc.scalar.dma_start(bk_sb[:, :], bk2[:, :])
        for ci in range(NCT):
            t_ = po.tile([128, DS], BF16, tag=f"wq{ci}", name=f"wq{ci}")
            nc.scalar.dma_start(t_[:, :], wq[128 * ci : 128 * (ci + 1), :])
            wq_sb.append(t_)
            t_ = po.tile([128, DS], BF16, tag=f"wk{ci}", name=f"wk{ci}")
            nc.scalar.dma_start(t_[:, :], wk[128 * ci : 128 * (ci + 1), :])
            wk_sb.append(t_)
        # vp ones columns via on-chip memset (no 1MB DMA)
        vp = po.tile([128, HPC * VW], BF16, tag="vp")
        nc.vector.memset(vp[:, :], 1.0)
        # gpsimd queue: mask first (priming + qc0 diagonals need it early)
        mask_sb = po.tile([128, 512], BF16, tag="mask")
        nc.gpsimd.dma_start(mask_sb[:, :], maskd[:, :])
        bv_sb = po.tile([128, DS], F32, tag="bv")
        nc.gpsimd.dma_start(bv_sb[:, :], bv_bc[:, :])
        ones_t = po.tile([128, 64], F32R, tag="ones")
        nc.gpsimd.dma_start(ones_t[:, :], onesd[:, :].bitcast(F32R))
        bo_sb = po.tile([128, DS], F32, tag="bo")
        nc.gpsimd.dma_start(bo_sb[:, :], bo_bc[:, :])
        # scalar queue (continued): wv, wo
        for ci in range(NCT):
            t_ = po.tile([128, DS], BF16, tag=f"wv{ci}", name=f"wv{ci}")
            nc.scalar.dma_start(t_[:, :], wv[128 * ci : 128 * (ci + 1), :])
            wv_sb.append(t_)
        for ci in range(2 * NCT):
            t_ = po.tile([128, DS], BF16, tag=f"wo{ci}", name=f"wo{ci}")
            nc.scalar.dma_start(t_[:, :], wo[128 * ci : 128 * (ci + 1), :])
            wo_sb.append(t_)

        # ---- persistent outputs of the projection stage -----------------
        qT_sb = [po.tile([128, T], BF16, tag=f"qT{m}", name=f"qT{m}") for m in range(2)]
        kT_sb = [po.tile([128, T], BF16, tag=f"kT{m}", name=f"kT{m}") for m in range(2)]
        attn_sb = {}  # (qc, pair_idx) -> [64, 1024] (two heads side by side)
        for q in range(NQC):
            for pi in range(2):
                attn_sb[(q, pi)] = po.tile(
                    [64, 1024], BF16, tag=f"at{q}_{pi}", name=f"at{q}_{pi}"
                )
        ao_t = {}  # (qc, ci) -> tile, allocated at AG-time from a 2-deep pool

        # ---- projection-chain emitters (each = one 8-MM PSUM chain) -----
        def qk_chain(dst, wsb, bsb, mt, tch):
            ps = psF.tile([128, 512], F32, tag="fps", name=f"qk{mt}_{tch}")
            for ci in range(NCT):
                nc.tensor.matmul(
                    ps[:, :],
                    wsb[ci][:, 128 * mt : 128 * (mt + 1)],
                    xT_sb[ci][:, 512 * tch : 512 * (tch + 1)],
                    start=(ci == 0),
                    stop=(ci == NCT - 1),
                )
            nc.vector.tensor_scalar_add(
                dst[mt][:, 512 * tch : 512 * (tch + 1)], ps[:, :],
                bsb[:, mt : mt + 1],
            )

        def v_chain(tt):
            ps = psF.tile([128, DS], F32, tag="fps", name=f"v{tt}")
            for ci in range(NCT):
                nc.tensor.matmul(
                    ps[:, :],
                    xT_sb[ci][:, 128 * tt : 128 * (tt + 1)],
                    wv_sb[ci][:, :],
                    start=(ci == 0),
                    stop=(ci == NCT - 1),
                )
            # V + bv into the per-head 65-col strided layout (ones col kept)
            dst = vp[:, :].rearrange(
                f"p (h t d) -> p h t d", h=HPC, t=NKT, d=65
            )[:, :, tt, 0:64]
            src = ps[:, :].rearrange(f"p (h d) -> p h d", h=HPC, d=64)
            bvv = bv_sb[:, :].rearrange(f"p (h d) -> p h d", h=HPC, d=64)
            nc.vector.tensor_tensor(dst, src, bvv, ALU.add)

        def out_chain(qc, tj):
            ps = psF.tile([128, DS], F32, tag="fps", name=f"out{qc}_{tj}")
            for k in range(2 * NCT):
                if qc < 3:
                    src_t, wi = ao_t[(qc, k)], k
                    cof = 512 * qc if qc < 2 else 0
                else:
                    pi, ci = k // NCT, k % NCT
                    src_t, wi = ao_t[(3, pi, ci)], 2 * ci + pi
                    cof = 0
                nc.tensor.matmul(
                    ps[:, :],
                    src_t[:, cof + 128 * tj : cof + 128 * (tj + 1)],
                    wo_sb[wi][:, :],
                    start=(k == 0),
                    stop=(k == 2 * NCT - 1),
                )
            osb = posb.tile([128, DS], F32, tag="osb", name=f"ou{qc}_{tj}")
            nc.vector.tensor_tensor(osb[:, :], ps[:, :], bo_sb[:, :], ALU.add)
            nc.sync.dma_start(
                out_shard[512 * qc + 128 * tj : 512 * qc + 128 * (tj + 1), :],
                osb[:, :],
            )

        fillers = []  # (min_step, emit_fn)
        fillers.append((0, lambda: qk_chain(kT_sb, wk_sb, bk_sb, 1, 0)))
        fillers.append((0, lambda: qk_chain(qT_sb, wq_sb, bq_sb, 1, 0)))
        fillers.append((0, lambda: v_chain(4)))
        fillers.append((0, lambda: v_chain(5)))
        for tch in (1, 2, 3):
            fillers.append((0, lambda m=0, t=tch: qk_chain(kT_sb, wk_sb, bk_sb, m, t)))
            fillers.append((0, lambda m=1, t=tch: qk_chain(kT_sb, wk_sb, bk_sb, m, t)))
            fillers.append((0, lambda m=0, t=tch: qk_chain(qT_sb, wq_sb, bq_sb, m, t)))
            fillers.append((0, lambda m=1, t=tch: qk_chain(qT_sb, wq_sb, bq_sb, m, t)))
            fillers.append((0, lambda t=4 * tch + 2: v_chain(t)))
            fillers.append((0, lambda t=4 * tch + 3: v_chain(t)))
            if tch < 3:
                fillers.append((0, lambda t=4 * tch + 4: v_chain(t)))
                fillers.append((0, lambda t=4 * tch + 5: v_chain(t)))

        def pop_filler(step, n=1):
            for _ in range(n):
                for idx, (ms, fn) in enumerate(fillers):
                    if ms <= step:
                        fillers.pop(idx)
                        fn()
                        break
                else:
                    return

        # ---- upfront minimal projections for qc0 pair (0,1) -------------
        qk_chain(kT_sb, wk_sb, bk_sb, 0, 0)
        qk_chain(qT_sb, wq_sb, bq_sb, 0, 0)
        for tt in range(4):
            v_chain(tt)
        if BISECT_NO_FILLERS:
            while fillers:
                fillers.pop(0)[1]()
        st_bufs = [
            psS.tile([128, 1024], F32, tag=f"st{b}", name=f"stbuf{b}")
            for b in range(2)
        ]
        for b in range(2):  # prime full-width so trimmed writes never leave
            for i in range(2):  # uninitialized bytes for the batched exp read
                nc.tensor.matmul(
                    st_bufs[b][:, 512 * i : 512 * (i + 1)],
                    mask_sb[0:64, 0:128],
                    mask_sb[0:64, 0:512],
                    start=True,
                    stop=True,
                )

        # ---- attention helpers ------------------------------------------
        def pv(qc, hp, kt, pTs, ops, nkt):
            pT, w = pTs.pop(kt)
            q0 = 512 - w  # col offset within the 512-q chunk
            for i, h in enumerate((hp, hp + 1)):
                nc.tensor.matmul(
                    ops[h][:, q0:512],
                    vp[:, VW * h + 65 * kt : VW * h + 65 * kt + 65],
                    pT[:, 512 * i : 512 * i + w],
                    start=(kt == 0),
                    stop=(kt == nkt - 1),
                )

        OUT_GATE = {0: 52, 1: 58, 2: 74}  # attention step gating outproj(qc)

        def emit_ag_pair(qc, pi):
            # one DMA moves both heads of the pair into the gather input
            if qc < 2:
                dst = agi01[128 * pi : 128 * (pi + 1), 512 * qc : 512 * (qc + 1)]
            elif qc == 2:
                dst = agi2[128 * pi : 128 * (pi + 1), :]
            else:
                dst = agi3[pi][:, :]
            nc.gpsimd.dma_start(
                dst.rearrange("(h d) q -> d h q", h=2, d=64),
                attn_sb[(qc, pi)][:, :].rearrange("d (h q) -> d h q", h=2),
            )
            if (qc, pi) == (1, 1):
                nc.gpsimd.collective_compute(
                    "AllGather", ALU.bypass, replica_groups=GROUPS,
                    ins=[agi01[:, :]], outs=[ago01[:, :]],
                )
                for ci in range(2 * NCT):
                    t_ = pao.tile([128, 1024], BF16, tag=f"aoA{ci}", bufs=1,
                                  name=f"ao01_{ci}")
                    nc.sync.dma_start(
                        t_[:, :], ago01[128 * ci : 128 * (ci + 1), :]
                    )
                    ao_t[(0, ci)] = t_
                    ao_t[(1, ci)] = t_
                for q in (0, 1):
                    for tj in range(4):
                        fillers.append(
                            (OUT_GATE[q], lambda q2=q, t=tj: out_chain(q2, t))
                        )
            elif (qc, pi) == (2, 1):
                nc.gpsimd.collective_compute(
                    "AllGather", ALU.bypass, replica_groups=GROUPS,
                    ins=[agi2[:, :]], outs=[ago2[:, :]],
                )
                for ci in range(2 * NCT):
                    t_ = pao.tile([128, 512], BF16, tag=f"ao{ci}",
                                  name=f"ao2_{ci}")
                    nc.sync.dma_start(
                        t_[:, :], ago2[128 * ci : 128 * (ci + 1), :]
                    )
                    ao_t[(2, ci)] = t_
                for tj in range(4):
                    fillers.append(
                        (OUT_GATE[2], lambda t=tj: out_chain(2, t))
                    )
            elif qc == 3:
                # qc3: gather each pair as soon as it completes; pair pi's
                # rows interleave as wo c-tiles 2*ci+pi in the contraction
                nc.gpsimd.collective_compute(
                    "AllGather", ALU.bypass, replica_groups=GROUPS,
                    ins=[agi3[pi][:, :]], outs=[ago3[pi][:, :]],
                )
                for ci in range(NCT):
                    t_ = pao.tile([128, 512], BF16, tag=f"ao{2 * ci + pi}",
                                  name=f"ao3{pi}_{ci}")
                    nc.sync.dma_start(
                        t_[:, :], ago3[pi][128 * ci : 128 * (ci + 1), :]
                    )
                    ao_t[(3, pi, ci)] = t_

        # ---- normalize: divide O' rows 0..63 by row-sums (row 64) -------
        pend = []  # (qc, pi, obs, rc) normalize front-halves awaiting bc+mult

        def norm_front(ops_pair, hp, qc):
            # copy O'+sums of both heads to SBUF (frees the PSUM banks), then
            # one batched 1/sums on DVE at [128, 8] via two SBUF<->SBUF DMAs
            pi = hp // 2
            ob = pn.tile([65, 1024], F32, tag="ob", name=f"ob{hp}_{qc}")
            for i, h in enumerate((hp, hp + 1)):
                nc.vector.tensor_copy(
                    ob[:, 512 * i : 512 * (i + 1)], ops_pair[h][:, :]
                )
            rs4 = pn.tile([128, 8], F32, tag="rs4", name=f"rs4{hp}_{qc}")
            nc.gpsimd.dma_start(rs4[:, :], ob[64:65, :])
            rr4 = pn.tile([128, 8], F32R, tag="rr4", name=f"rr4{hp}_{qc}")
            with nc.allow_low_precision(reason="f32r matmul feed"):
                nc.vector.reciprocal(rr4[:, :], rs4[:, :])
            rc = pn.tile([65, 1024], F32R, tag="rc", name=f"rc{hp}_{qc}")
            nc.gpsimd.dma_start(rc[64:65, :], rr4[:, :])
            pend.append((qc, pi, ob, rc))

        def norm_back():
            for qc, pi, ob, rc in pend:
                for i in range(2):
                    bc = psF.tile([64, 512], F32, tag="fps",
                                  name=f"bc{qc}_{pi}_{i}")
                    nc.tensor.matmul(
                        bc[:, :], _r(ones_t[64:65, :]),
                        rc[64:65, 512 * i : 512 * (i + 1)],
                        start=True, stop=True,
                    )
                    nc.vector.tensor_tensor(
                        attn_sb[(qc, pi)][:, 512 * i : 512 * (i + 1)],
                        ob[0:64, 512 * i : 512 * (i + 1)], bc[:, :], ALU.mult,
                    )
                emit_ag_pair(qc, pi)
            pend.clear()

        # ---- causal attention (S^T orientation), paired heads -----------
        gstep = 0
        for qc in range(NQC):
            nkt = 4 * qc + 4
            for hp in (0, 2):
                mt = hp // 2
                ops = {
                    h: psO.tile([65, 512], F32, tag=f"ops{h % 2}",
                                name=f"op_q{qc}h{h}")
                    for h in (hp, hp + 1)
                }
                pTs = {}
                for kt in range(nkt):
                    dm = kt - 4 * qc
                    w = 512 - 128 * dm if dm >= 0 else 512
                    q0 = 512 * qc + (512 - w)
                    # S pair: concurrent row-strip matmuls into one 2-bank
                    # PSUM tile (head A cols [0:w], head B cols [512:512+w])
                    st = st_bufs[gstep % 2]
                    for i, h in enumerate((hp, hp + 1)):
                        pof = 64 * (h % 2)
                        nc.tensor.matmul(
                            st[:, 512 * i : 512 * i + w],
                            kT_sb[mt][pof : pof + 64, 128 * kt : 128 * (kt + 1)],
                            qT_sb[mt][pof : pof + 64, q0 : q0 + w],
                            start=True,
                            stop=True,
                        )
                    pT = pw.tile([128, 1024], BF16, tag="pT",
                                 name=f"p{qc}_{hp}_{kt}")
                    nc.scalar.activation(
                        pT[:, 0 : 512 + w], st[:, 0 : 512 + w], AF.Exp,
                        scale=0.125,
                    )
                    if dm >= 0:  # diagonal: causal mask (k <= q')
                        for i in range(2):
                            nc.vector.tensor_tensor(
                                pT[:, 512 * i : 512 * i + w],
                                pT[:, 512 * i : 512 * i + w],
                                mask_sb[:, 0:w],
                                ALU.mult,
                            )
                    pTs[kt] = (pT, w)
                    if kt > 0:
                        pv(qc, hp, kt - 1, pTs, ops, nkt)
                    if kt == 1:
                        norm_back()  # prev pair's bcast+mult (+AG when ready)
                    pop_filler(gstep, 2 if gstep < 16 else 1)
                    gstep += 1
                pv(qc, hp, nkt - 1, pTs, ops, nkt)
                norm_front(ops, hp, qc)
        pop_filler(10**9, 2)
        norm_back()

        # ---- tail: drain remaining fillers, then qc3 output projection --
        while fillers:
            _, fn = fillers.pop(0)
            fn()
        for tj in range(4):
            out_chain(3, tj)


def _build_program():
    nc = bacc.Bacc(
        "TRN2",
        target_bir_lowering=False,
        debug=False,
        num_devices=NCORES,
    )
    xT = nc.dram_tensor("xT", [C, T], BF16, kind="ExternalInput")
    wq = nc.dram_tensor("wq", [C, DS], BF16, kind="ExternalInput")
    wk = nc.dram_tensor("wk", [C, DS], BF16, kind="ExternalInput")
    wv = nc.dram_tensor("wv", [C, DS], BF16, kind="ExternalInput")
    wo = nc.dram_tensor("wo", [2 * C, DS], BF16, kind="ExternalInput")
    bq2 = nc.dram_tensor("bq2", [128, 2], F32, kind="ExternalInput")
    bk2 = nc.dram_tensor("bk2", [128, 2], F32, kind="ExternalInput")
    bv_bc = nc.dram_tensor("bv_bc", [128, DS], F32, kind="ExternalInput")
    bo_bc = nc.dram_tensor("bo_bc", [128, DS], F32, kind="ExternalInput")
    maskd = nc.dram_tensor("maskd", [128, 512], BF16, kind="ExternalInput")
    onesd = nc.dram_tensor("onesd", [128, 64], F32, kind="ExternalInput")
    out_shard = nc.dram_tensor("out_shard", [T, DS], F32, kind="ExternalOutput")
    io = (xT, wq, wk, wv, wo, bq2, bk2, bv_bc, bo_bc, maskd, onesd,
          out_shard)
    with tile.TileContext(nc) as tc:
        _emit(nc, tc, io)
    nc.compile()
    return nc


def _pad_wo(wo_slice, b):
    out = np.zeros((2 * C, DS), np.float32)
    out[C * b : C * (b + 1), :] = wo_slice
    return out.astype(NPBF)


def _make_mask():
    # lower-triangular multiplicative mask: visible iff k <= q'
    k = np.arange(128, dtype=np.int64)[:, None]
    q = np.arange(512, dtype=np.int64)[None, :]
    return (k <= q).astype(np.float32).astype(NPBF)


def _make_in_maps(x, Wq, bq, Wk, bk, Wv, bv, Wo, bo):
    mask = _make_mask()
    in_maps = []
    for c in range(NCORES):
        b, g = c // GP, c % GP
        hs = slice(DS * g, DS * (g + 1))
        in_maps.append(
            {
                "xT": np.ascontiguousarray(x[b].T).astype(NPBF),
                "wq": np.ascontiguousarray(Wq[:, hs]).astype(NPBF),
                "wk": np.ascontiguousarray(Wk[:, hs]).astype(NPBF),
                "wv": np.ascontiguousarray(Wv[:, hs]).astype(NPBF),
                "wo": _pad_wo(Wo[:, hs], b),
                "bq2": np.ascontiguousarray(bq[hs].reshape(2, 128).T),
                "bk2": np.ascontiguousarray(bk[hs].reshape(2, 128).T),
                "bv_bc": np.tile(bv[hs][None, :], (128, 1)).astype(np.float32),
                "bo_bc": np.tile(bo[hs][None, :], (128, 1)).astype(np.float32),
                "maskd": mask,
                "onesd": np.ones((128, 64), np.float32),
            }
        )
    return in_maps


def kernel(x, Wq, bq, Wk, bk, Wv, bv, Wo, bo, _trace=False, _trace_cores=None):
    global _PROG, LAST_RESULTS
    x = np.asarray(x, np.float32)
    Wq, bq = np.asarray(Wq, np.float32), np.asarray(bq, np.float32)
    Wk, bk = np.asarray(Wk, np.float32), np.asarray(bk, np.float32)
    Wv, bv = np.asarray(Wv, np.float32), np.asarray(bv, np.float32)
    Wo, bo = np.asarray(Wo, np.float32), np.asarray(bo, np.float32)

    if _PROG is None:
        _PROG = _build_program()
    nc = _PROG

    in_maps = _make_in_maps(x, Wq, bq, Wk, bk, Wv, bv, Wo, bo)

    kw = {}
    if _trace:
        kw["trace"] = True
        if _trace_cores is not None:
            kw["trace_cores"] = _trace_cores
    res = bass_utils.run_bass_kernel_spmd(nc, in_maps, list(range(NCORES)), **kw)
    LAST_RESULTS = res

    out = np.empty((B, T, C), np.float32)
    for c in range(NCORES):
        b, g = c // GP, c % GP
        out[b, :, DS * g : DS * (g + 1)] = res.results[c]["out_shard"]
    return out


# revision 38
# speedup vs baseline: 1.1594x; 1.1594x over previous
"""Causal multi-head attention (B=2, T=2048, C=1024, H=16) on 8 TRN2 cores.

Sharding: data-parallel over batch (2 groups of 4 cores), tensor-parallel
over heads within a group (4 heads / core). Each core:
  1. computes Q^T, K^T (layout [d, t]) and V (layout [t, d], bias folded
     in) for its heads from x[b]^T and its W column slices,
  2. runs causal attention in the S^T = K @ Q^T orientation; softmax sums
     come from a ones-column appended to V; exp is batched over both heads
     of a pair (one ACT per k-step over a 2-bank PSUM tile); diagonal
     k-tiles are trimmed to q >= 128*dm,
  3. AllGathers the per-head attention outputs per q-chunk across the 4
     cores of its batch group,
  4. computes a 256-column slice of the output projection per q-chunk.
Host reassembles the 8 [2048, 256] shards into [2, 2048, 1024].

Scheduling: projection chains that are not needed upfront are emitted as
"fillers" between attention steps so the PE stream stays dense while the
scalar engine (exp) is the per-step bottleneck.
"""

import os
import sys

import numpy as np
import ml_dtypes

for _p in ("/opt/trn_rl_repo",):
    if os.path.isdir(_p) and _p not in sys.path:
        sys.path.insert(0, _p)

import concourse.bacc as bacc
import concourse.mybir as mybir
import concourse.tile as tile
from concourse import bass_utils

B, T, C, H, D = 2, 2048, 1024, 16, 64
NCORES = 8
GP = 4              # cores per batch group
HPC = H // GP       # heads per core = 4
DS = HPC * D        # per-core head-dim slice = 256
NCT = C // 128      # c-tiles = 8
NQC = T // 512      # q-chunks = 4
NKT = T // 128      # k-tiles = 16
VW = NKT * 65       # vp row width per head (64 V cols + ones col per k-tile)
BISECT_NO_FILLERS = False

F32 = mybir.dt.float32
F32R = mybir.dt.float32r
BF16 = mybir.dt.bfloat16
AF = mybir.ActivationFunctionType
ALU = mybir.AluOpType
NPBF = ml_dtypes.bfloat16

_PROG = None
LAST_RESULTS = None  # BassKernelResults of the most recent run (for test.py)


def _r(ap):
    return ap.bitcast(F32R)


def _emit(nc, tc, io):
    (xT, wq, wk, wv, wo, bq2, bk2, bv_bc, bo_bc, maskd, onesd,
     out_shard) = io

    agi = [nc.dram_tensor(f"agi{q}", [DS, 512], BF16) for q in range(3)]
    ago = [
        nc.dram_tensor(f"ago{q}", [NCORES * DS, 512], BF16, addr_space="Shared")
        for q in range(3)
    ]
    agi3 = [nc.dram_tensor(f"agi3{p}", [128, 512], BF16) for p in range(2)]
    ago3 = [
        nc.dram_tensor(f"ago3{p}", [NCORES * 128, 512], BF16, addr_space="Shared")
        for p in range(2)
    ]
    GROUPS = [list(range(NCORES))]

    with (
        tc.tile_pool(name="per", bufs=1) as po,
        tc.tile_pool(name="pT", bufs=4) as pw,
        tc.tile_pool(name="nrm", bufs=4) as pn,
        tc.tile_pool(name="osb", bufs=3) as posb,
        tc.tile_pool(name="pao", bufs=2) as pao,
        tc.tile_pool(name="psS", bufs=1, space="PSUM") as psS,
        tc.tile_pool(name="psO", bufs=1, space="PSUM") as psO,
        tc.tile_pool(name="psF", bufs=2, space="PSUM") as psF,
    ):
        # ---- input loads, spread over per-engine DMA queues -------------
        # sync: xT in column halves (first half unblocks qc0 work early)
        xT_sb = []
        for ci in range(NCT):
            t_ = po.tile([128, T], BF16, tag=f"xt{ci}", name=f"xt{ci}")
            xT_sb.append(t_)
        for th in range(2):
            for ci in range(NCT):
                nc.sync.dma_start(
                    xT_sb[ci][:, 1024 * th : 1024 * (th + 1)],
                    xT[128 * ci : 128 * (ci + 1), 1024 * th : 1024 * (th + 1)],
                )
        # scalar queue: wq, wk + biases
        wq_sb, wk_sb, wv_sb, wo_sb = [], [], [], []
        bq_sb = po.tile([128, 2], F32, tag="bq")
        nc.scalar.dma_start(bq_sb[:, :], bq2[:, :])
        bk_sb = po.tile([128, 2], F32, tag="bk")
        nc.scalar.dma_start(bk_sb[:, :], bk2[:, :])
        for ci in range(NCT):
            t_ = po.tile([128, DS], BF16, tag=f"wq{ci}", name=f"wq{ci}")
            nc.scalar.dma_start(t_[:, :], wq[128 * ci : 128 * (ci + 1), :])
            wq_sb.append(t_)
            t_ = po.tile([128, DS], BF16, tag=f"wk{ci}", name=f"wk{ci}")
            nc.scalar.dma_start(t_[:, :], wk[128 * ci : 128 * (ci + 1), :])
            wk_sb.append(t_)
        # vp ones columns via on-chip memset (no 1MB DMA)
        vp = po.tile([128, HPC * VW], BF16, tag="vp")
        nc.vector.memset(vp[:, :], 1.0)
        # gpsimd queue: mask first (priming + qc0 diagonals need it early)
        mask_sb = po.tile([128, 512], BF16, tag="mask")
        nc.gpsimd.dma_start(mask_sb[:, :], maskd[:, :])
        bv_sb = po.tile([128, DS], F32, tag="bv")
        nc.gpsimd.dma_start(bv_sb[:, :], bv_bc[:, :])
        ones_t = po.tile([128, 64], F32R, tag="ones")
        nc.gpsimd.dma_start(ones_t[:, :], onesd[:, :].bitcast(F32R))
        bo_sb = po.tile([128, DS], F32, tag="bo")
        nc.gpsimd.dma_start(bo_sb[:, :], bo_bc[:, :])
        # scalar queue (continued): wv, wo
        for ci in range(NCT):
            t_ = po.tile([128, DS], BF16, tag=f"wv{ci}", name=f"wv{ci}")
            nc.scalar.dma_start(t_[:, :], wv[128 * ci : 128 * (ci + 1), :])
            wv_sb.append(t_)
        for ci in range(2 * NCT):
            t_ = po.tile([128, DS], BF16, tag=f"wo{ci}", name=f"wo{ci}")
            nc.scalar.dma_start(t_[:, :], wo[128 * ci : 128 * (ci + 1), :])
            wo_sb.append(t_)

        # ---- persistent outputs of the projection stage -----------------
        qT_sb = [po.tile([128, T], BF16, tag=f"qT{m}", name=f"qT{m}") for m in range(2)]
        kT_sb = [po.tile([128, T], BF16, tag=f"kT{m}", name=f"kT{m}") for m in range(2)]
        attn_sb = {}  # (qc, pair_idx) -> [64, 1024] (two heads side by side)
        for q in range(NQC):
            for pi in range(2):
                attn_sb[(q, pi)] = po.tile(
                    [64, 1024], BF16, tag=f"at{q}_{pi}", name=f"at{q}_{pi}"
                )
        ao_t = {}  # (qc, ci) -> tile, allocated at AG-time from a 2-deep pool

        # ---- projection-chain emitters (each = one 8-MM PSUM chain) -----
        def qk_chain(dst, wsb, bsb, mt, tch):
            ps = psF.tile([128, 512], F32, tag="fps", name=f"qk{mt}_{tch}")
            for ci in range(NCT):
                nc.tensor.matmul(
                    ps[:, :],
                    wsb[ci][:, 128 * mt : 128 * (mt + 1)],
                    xT_sb[ci][:, 512 * tch : 512 * (tch + 1)],
                    start=(ci == 0),
                    stop=(ci == NCT - 1),
                )
            nc.vector.tensor_scalar_add(
                dst[mt][:, 512 * tch : 512 * (tch + 1)], ps[:, :],
                bsb[:, mt : mt + 1],
            )

        def v_chain(tt):
            ps = psF.tile([128, DS], F32, tag="fps", name=f"v{tt}")
            for ci in range(NCT):
                nc.tensor.matmul(
                    ps[:, :],
                    xT_sb[ci][:, 128 * tt : 128 * (tt + 1)],
                    wv_sb[ci][:, :],
                    start=(ci == 0),
                    stop=(ci == NCT - 1),
                )
            # V + bv into the per-head 65-col strided layout (ones col kept)
            dst = vp[:, :].rearrange(
                f"p (h t d) -> p h t d", h=HPC, t=NKT, d=65
            )[:, :, tt, 0:64]
            src = ps[:, :].rearrange(f"p (h d) -> p h d", h=HPC, d=64)
            bvv = bv_sb[:, :].rearrange(f"p (h d) -> p h d", h=HPC, d=64)
            nc.vector.tensor_tensor(dst, src, bvv, ALU.add)

        def out_chain(qc, tj):
            ps = psF.tile([128, DS], F32, tag="fps", name=f"out{qc}_{tj}")
            for k in range(2 * NCT):
                if qc < 3:
                    src_t, wi = ao_t[(qc, k)], k
                else:
                    pi, ci = k // NCT, k % NCT
                    src_t, wi = ao_t[(3, pi, ci)], 2 * ci + pi
                nc.tensor.matmul(
                    ps[:, :],
                    src_t[:, 128 * tj : 128 * (tj + 1)],
                    wo_sb[wi][:, :],
                    start=(k == 0),
                    stop=(k == 2 * NCT - 1),
                )
            osb = posb.tile([128, DS], F32, tag="osb", name=f"ou{qc}_{tj}")
            nc.vector.tensor_tensor(osb[:, :], ps[:, :], bo_sb[:, :], ALU.add)
            nc.sync.dma_start(
                out_shard[512 * qc + 128 * tj : 512 * qc + 128 * (tj + 1), :],
                osb[:, :],
            )

        fillers = []  # (min_step, emit_fn)
        fillers.append((0, lambda: qk_chain(kT_sb, wk_sb, bk_sb, 1, 0)))
        fillers.append((0, lambda: qk_chain(qT_sb, wq_sb, bq_sb, 1, 0)))
        fillers.append((0, lambda: v_chain(4)))
        fillers.append((0, lambda: v_chain(5)))
        for tch in (1, 2, 3):
            fillers.append((0, lambda m=0, t=tch: qk_chain(kT_sb, wk_sb, bk_sb, m, t)))
            fillers.append((0, lambda m=1, t=tch: qk_chain(kT_sb, wk_sb, bk_sb, m, t)))
            fillers.append((0, lambda m=0, t=tch: qk_chain(qT_sb, wq_sb, bq_sb, m, t)))
            fillers.append((0, lambda m=1, t=tch: qk_chain(qT_sb, wq_sb, bq_sb, m, t)))
            fillers.append((0, lambda t=4 * tch + 2: v_chain(t)))
            fillers.append((0, lambda t=4 * tch + 3: v_chain(t)))
            if tch < 3:
                fillers.append((0, lambda t=4 * tch + 4: v_chain(t)))
                fillers.append((0, lambda t=4 * tch + 5: v_chain(t)))

        def pop_filler(step, n=1):
            for _ in range(n):
                for idx, (ms, fn) in enumerate(fillers):
                    if ms <= step:
                        fillers.pop(idx)
                        fn()
                        break
                else:
                    return

        # ---- upfront minimal projections for qc0 pair (0,1) -------------
        qk_chain(kT_sb, wk_sb, bk_sb, 0, 0)
        qk_chain(qT_sb, wq_sb, bq_sb, 0, 0)
        for tt in range(4):
            v_chain(tt)
        if BISECT_NO_FILLERS:
            while fillers:
                fillers.pop(0)[1]()
        st_bufs = [
            psS.tile([128, 1024], F32, tag=f"st{b}", name=f"stbuf{b}")
            for b in range(2)
        ]
        for b in range(2):  # prime full-width so trimmed writes never leave
            for i in range(2):  # uninitialized bytes for the batched exp read
                nc.tensor.matmul(
                    st_bufs[b][:, 512 * i : 512 * (i + 1)],
                    mask_sb[0:64, 0:128],
                    mask_sb[0:64, 0:512],
                    start=True,
                    stop=True,
                )

        # ---- attention helpers ------------------------------------------
        def pv(qc, hp, kt, pTs, ops, nkt):
            pT, w = pTs.pop(kt)
            q0 = 512 - w  # col offset within the 512-q chunk
            for i, h in enumerate((hp, hp + 1)):
                nc.tensor.matmul(
                    ops[h][:, q0:512],
                    vp[:, VW * h + 65 * kt : VW * h + 65 * kt + 65],
                    pT[:, 512 * i : 512 * i + w],
                    start=(kt == 0),
                    stop=(kt == nkt - 1),
                )

        OUT_GATE = {0: 48, 1: 60, 2: 76}  # attention step gating outproj(qc)

        def emit_ag_pair(qc, pi):
            # one DMA moves both heads of the pair into the gather input
            if qc < 3:
                dst = agi[qc][128 * pi : 128 * (pi + 1), :]
            else:
                dst = agi3[pi][:, :]
            nc.gpsimd.dma_start(
                dst.rearrange("(h d) q -> d h q", h=2, d=64),
                attn_sb[(qc, pi)][:, :].rearrange("d (h q) -> d h q", h=2),
            )
            if qc < 3 and pi == 1:
                nc.gpsimd.collective_compute(
                    "AllGather", ALU.bypass, replica_groups=GROUPS,
                    ins=[agi[qc][:, :]], outs=[ago[qc][:, :]],
                )
                for ci in range(2 * NCT):
                    t_ = pao.tile([128, 512], BF16, tag=f"ao{ci}",
                                  name=f"ao{qc}_{ci}")
                    nc.sync.dma_start(
                        t_[:, :], ago[qc][128 * ci : 128 * (ci + 1), :]
                    )
                    ao_t[(qc, ci)] = t_
                for tj in range(4):
                    fillers.append(
                        (OUT_GATE[qc], lambda q=qc, t=tj: out_chain(q, t))
                    )
            elif qc == 3:
                # qc3: gather each pair as soon as it completes; pair pi's
                # rows interleave as wo c-tiles 2*ci+pi in the contraction
                nc.gpsimd.collective_compute(
                    "AllGather", ALU.bypass, replica_groups=GROUPS,
                    ins=[agi3[pi][:, :]], outs=[ago3[pi][:, :]],
                )
                for ci in range(NCT):
                    t_ = pao.tile([128, 512], BF16, tag=f"ao{2 * ci + pi}",
                                  name=f"ao3{pi}_{ci}")
                    nc.sync.dma_start(
                        t_[:, :], ago3[pi][128 * ci : 128 * (ci + 1), :]
                    )
                    ao_t[(3, pi, ci)] = t_

        # ---- normalize: divide O' rows 0..63 by row-sums (row 64) -------
        pend = []  # (qc, pi, obs, rc) normalize front-halves awaiting bc+mult

        def norm_front(ops_pair, hp, qc):
            # copy O'+sums of both heads to SBUF (frees the PSUM banks), then
            # one batched 1/sums on DVE at [128, 8] via two SBUF<->SBUF DMAs
            pi = hp // 2
            ob = pn.tile([65, 1024], F32, tag="ob", name=f"ob{hp}_{qc}")
            for i, h in enumerate((hp, hp + 1)):
                nc.vector.tensor_copy(
                    ob[:, 512 * i : 512 * (i + 1)], ops_pair[h][:, :]
                )
            rs4 = pn.tile([128, 8], F32, tag="rs4", name=f"rs4{hp}_{qc}")
            nc.gpsimd.dma_start(rs4[:, :], ob[64:65, :])
            rr4 = pn.tile([128, 8], F32R, tag="rr4", name=f"rr4{hp}_{qc}")
            with nc.allow_low_precision(reason="f32r matmul feed"):
                nc.vector.reciprocal(rr4[:, :], rs4[:, :])
            rc = pn.tile([65, 1024], F32R, tag="rc", name=f"rc{hp}_{qc}")
            nc.gpsimd.dma_start(rc[64:65, :], rr4[:, :])
            pend.append((qc, pi, ob, rc))

        def norm_back():
            for qc, pi, ob, rc in pend:
                for i in range(2):
                    bc = psF.tile([64, 512], F32, tag="fps",
                                  name=f"bc{qc}_{pi}_{i}")
                    nc.tensor.matmul(
                        bc[:, :], _r(ones_t[64:65, :]),
                        rc[64:65, 512 * i : 512 * (i + 1)],
                        start=True, stop=True,
                    )
                    nc.vector.tensor_tensor(
                        attn_sb[(qc, pi)][:, 512 * i : 512 * (i + 1)],
                        ob[0:64, 512 * i : 512 * (i + 1)], bc[:, :], ALU.mult,
                    )
                emit_ag_pair(qc, pi)
            pend.clear()

        # ---- causal attention (S^T orientation), paired heads -----------
        gstep = 0
        for qc in range(NQC):
            nkt = 4 * qc + 4
            for hp in (0, 2):
                mt = hp // 2
                ops = {
                    h: psO.tile([65, 512], F32, tag=f"ops{h % 2}",
                                name=f"op_q{qc}h{h}")
                    for h in (hp, hp + 1)
                }
                pTs = {}
                for kt in range(nkt):
                    dm = kt - 4 * qc
                    w = 512 - 128 * dm if dm >= 0 else 512
                    q0 = 512 * qc + (512 - w)
                    # S pair: concurrent row-strip matmuls into one 2-bank
                    # PSUM tile (head A cols [0:w], head B cols [512:512+w])
                    st = st_bufs[gstep % 2]
                    for i, h in enumerate((hp, hp + 1)):
                        pof = 64 * (h % 2)
                        nc.tensor.matmul(
                            st[:, 512 * i : 512 * i + w],
                            kT_sb[mt][pof : pof + 64, 128 * kt : 128 * (kt + 1)],
                            qT_sb[mt][pof : pof + 64, q0 : q0 + w],
                            start=True,
                            stop=True,
                        )
                    pT = pw.tile([128, 1024], BF16, tag="pT",
                                 name=f"p{qc}_{hp}_{kt}")
                    nc.scalar.activation(
                        pT[:, 0 : 512 + w], st[:, 0 : 512 + w], AF.Exp,
                        scale=0.125,
                    )
                    if dm >= 0:  # diagonal: causal mask (k <= q')
                        for i in range(2):
                            nc.vector.tensor_tensor(
                                pT[:, 512 * i : 512 * i + w],
                                pT[:, 512 * i : 512 * i + w],
                                mask_sb[:, 0:w],
                                ALU.mult,
                            )
                    pTs[kt] = (pT, w)
                    if kt > 0:
                        pv(qc, hp, kt - 1, pTs, ops, nkt)
                    if kt == 1:
                        norm_back()  # prev pair's bcast+mult (+AG when ready)
                    pop_filler(gstep, 2 if gstep < 16 else 1)
                    gstep += 1
                pv(qc, hp, nkt - 1, pTs, ops, nkt)
                norm_front(ops, hp, qc)
        pop_filler(10**9, 2)
        norm_back()

        # ---- tail: drain remaining fillers, then qc3 output projection --
        while fillers:
            _, fn = fillers.pop(0)
            fn()
        for tj in range(4):
            out_chain(3, tj)


def _build_program():
    nc = bacc.Bacc(
        "TRN2",
        target_bir_lowering=False,
        debug=False,
        num_devices=NCORES,
    )
    xT = nc.dram_tensor("xT", [C, T], BF16, kind="ExternalInput")
    wq = nc.dram_tensor("wq", [C, DS], BF16, kind="ExternalInput")
    wk = nc.dram_tensor("wk", [C, DS], BF16, kind="ExternalInput")
    wv = nc.dram_tensor("wv", [C, DS], BF16, kind="ExternalInput")
    wo = nc.dram_tensor("wo", [2 * C, DS], BF16, kind="ExternalInput")
    bq2 = nc.dram_tensor("bq2", [128, 2], F32, kind="ExternalInput")
    bk2 = nc.dram_tensor("bk2", [128, 2], F32, kind="ExternalInput")
    bv_bc = nc.dram_tensor("bv_bc", [128, DS], F32, kind="ExternalInput")
    bo_bc = nc.dram_tensor("bo_bc", [128, DS], F32, kind="ExternalInput")
    maskd = nc.dram_tensor("maskd", [128, 512], BF16, kind="ExternalInput")
    onesd = nc.dram_tensor("onesd", [128, 64], F32, kind="ExternalInput")
    out_shard = nc.dram_tensor("out_shard", [T, DS], F32, kind="ExternalOutput")
    io = (xT, wq, wk, wv, wo, bq2, bk2, bv_bc, bo_bc, maskd, onesd,
          out_shard)
    with tile.TileContext(nc) as tc:
        _emit(nc, tc, io)
    nc.compile()
    return nc


def _pad_wo(wo_slice, b):
    out = np.zeros((2 * C, DS), np.float32)
    out[C * b : C * (b + 1), :] = wo_slice
    return out.astype(NPBF)


def _make_mask():
    # lower-triangular multiplicative mask: visible iff k <= q'
    k = np.arange(128, dtype=np.int64)[:, None]
    q = np.arange(512, dtype=np.int64)[None, :]
    return (k <= q).astype(np.float32).astype(NPBF)


def _make_in_maps(x, Wq, bq, Wk, bk, Wv, bv, Wo, bo):
    mask = _make_mask()
    in_maps = []
    for c in range(NCORES):
        b, g = c // GP, c % GP
        hs = slice(DS * g, DS * (g + 1))
        in_maps.append(
            {
                "xT": np.ascontiguousarray(x[b].T).astype(NPBF),
                "wq": np.ascontiguousarray(Wq[:, hs]).astype(NPBF),
                "wk": np.ascontiguousarray(Wk[:, hs]).astype(NPBF),
                "wv": np.ascontiguousarray(Wv[:, hs]).astype(NPBF),
                "wo": _pad_wo(Wo[:, hs], b),
                "bq2": np.ascontiguousarray(bq[hs].reshape(2, 128).T),
                "bk2": np.ascontiguousarray(bk[hs].reshape(2, 128).T),
                "bv_bc": np.tile(bv[hs][None, :], (128, 1)).astype(np.float32),
                "bo_bc": np.tile(bo[hs][None, :], (128, 1)).astype(np.float32),
                "maskd": mask,
                "onesd": np.ones((128, 64), np.float32),
            }
        )
    return in_maps


def kernel(x, Wq, bq, Wk, bk, Wv, bv, Wo, bo, _trace=False, _trace_cores=None):
    global _PROG, LAST_RESULTS
    x = np.asarray(x, np.float32)
    Wq, bq = np.asarray(Wq, np.float32), np.asarray(bq, np.float32)
    Wk, bk = np.asarray(Wk, np.float32), np.asarray(bk, np.float32)
    Wv, bv = np.asarray(Wv, np.float32), np.asarray(bv, np.float32)
    Wo, bo = np.asarray(Wo, np.float32), np.asarray(bo, np.float32)

    if _PROG is None:
        _PROG = _build_program()
    nc = _PROG

    in_maps = _make_in_maps(x, Wq, bq, Wk, bk, Wv, bv, Wo, bo)

    kw = {}
    if _trace:
        kw["trace"] = True
        if _trace_cores is not None:
            kw["trace_cores"] = _trace_cores
    res = bass_utils.run_bass_kernel_spmd(nc, in_maps, list(range(NCORES)), **kw)
    LAST_RESULTS = res

    out = np.empty((B, T, C), np.float32)
    for c in range(NCORES):
        b, g = c // GP, c % GP
        out[b, :, DS * g : DS * (g + 1)] = res.results[c]["out_shard"]
    return out
